# revision 9
# baseline (speedup 1.0000x reference)
"""Trainium2 Bass kernel for a selective-SSM block (LN -> x_proj ->
softplus(dt_proj) -> diagonal SSM scan over L -> out_proj).

Sharding: 8 cores = 2 batches x 4 D-quarters. Each core runs the scan for its
512 channels over the full sequence. out_proj partials are summed on the host.

v2 design:
- L processed in 4 quarters of 1024 steps (pipelined A -> B -> C phases).
- Phase B chains G=8 state-index segments into ONE tensor_tensor_scan of
  ~8208 elements (the scan has ~4us fixed cost but ~0.3ns/elem marginal);
  state resets at segment boundaries via dA=0 (delta prepend columns hold
  +60000 so exp gives 0) and the cross-quarter carry is injected through a
  2-column [hc, hc] prepend in u.
- B/C rows are broadcast to 128 partitions by stride-0 DMA reads from a DRAM
  scratch (no PE/scalar broadcast work).
- y = sum_n C_n*h_n is accumulated by gpsimd DMA-accumulate (f16 sources into
  an f32 SBUF accumulator) - no compute-engine adds.
- f16 data throughout; LN stats and softplus intermediates in f32.
"""

import os
import sys

import numpy as np

try:
    import concourse.bass as bass
except ImportError:
    sys.path.insert(0, "/opt/trn_rl_repo")
    import concourse.bass as bass

import concourse.tile as tile
from concourse import mybir
from concourse.bass_utils import run_bass_kernel_spmd

F32 = mybir.dt.float32
F16 = mybir.dt.float16
AF = mybir.ActivationFunctionType
ALU = mybir.AluOpType

B, L, D, N, R = 2, 4096, 2048, 64, 128
P_PROJ = 2 * N + R  # 256
DQ = D // 4  # channels per core: 512
NCH = DQ // 128  # own d-chunks: 4
NCHALL = D // 128  # all d-chunks: 16
EPS = 1e-5

LQ = 1024  # quarter length
TB = 512  # phase A/C time block
G = 8  # state indices per scan instruction
SEG = LQ + 2  # segment length incl. 2-col carry prepend
NGRP = N // G

LAST_RESULTS = None  # BassKernelResults of the most recent run (for test.py)

_PROGRAM_CACHE = {}


def _build(nc, L_):
    NQ = L_ // LQ
    NTB = LQ // TB

    xT = nc.dram_tensor("xT", [D, L_], F16, kind="ExternalInput")
    wxF = nc.dram_tensor("wxF", [D, P_PROJ], F16, kind="ExternalInput")
    g0c = nc.dram_tensor("g0c", [128, 2], F32, kind="ExternalInput")
    c0c = nc.dram_tensor("c0c", [128, 2], F32, kind="ExternalInput")
    dtwT = nc.dram_tensor("dtwT", [R, DQ], F16, kind="ExternalInput")
    dtbc = nc.dram_tensor("dtbc", [128, NCH], F32, kind="ExternalInput")
    acols = nc.dram_tensor("acols", [128, NCH, N], F32, kind="ExternalInput")
    wbc = nc.dram_tensor("wbc", [128, NCH], F32, kind="ExternalInput")
    bbc = nc.dram_tensor("bbc", [128, NCH], F32, kind="ExternalInput")
    dpc = nc.dram_tensor("dpc", [128, NCH], F32, kind="ExternalInput")
    woT = nc.dram_tensor("woT", [DQ, D], F16, kind="ExternalInput")
    onesc = nc.dram_tensor("onesc", [128, 128], F16, kind="ExternalInput")
    idmc = nc.dram_tensor("idmc", [128, 128], F16, kind="ExternalInput")
    out_part = nc.dram_tensor("out_part", [D, L_], F32, kind="ExternalOutput")
    bcscr = nc.dram_tensor("bcscr", [128, L_], F16, kind="Internal")

    from contextlib import ExitStack

    with tile.TileContext(nc) as tc:
        with ExitStack() as stack:
            ep = lambda **kw: stack.enter_context(tc.tile_pool(**kw))
            single = ep(name="single", bufs=1)
            hcp = ep(name="hcp", bufs=1)
            qpool = ep(name="qpool", bufs=2)
            zpool = ep(name="zpool", bufs=2)
            gpp = ep(name="gpp", bufs=2)
            xin = ep(name="xin", bufs=2)
            xown = ep(name="xown", bufs=5)
            wpool = ep(name="wpool", bufs=2)
            wrk = ep(name="wrk", bufs=1)
            stats = ep(name="stats", bufs=1)
            stats2 = ep(name="stats2", bufs=1)
            bigA = ep(name="bigA", bufs=2)
            bigU = ep(name="bigU", bufs=2)
            bigH = ep(name="bigH", bufs=1)
            bcBp = ep(name="bcB", bufs=1)
            bcCp = ep(name="bcC", bufs=1)
            cpool = ep(name="cpool", bufs=2)
            wopool = ep(name="wopool", bufs=3)
            psum = ep(name="psum", bufs=1, space=bass.MemorySpace.PSUM)
            # --- constants ---
            ones128 = single.tile([128, 128], F16)
            nc.sync.dma_start(ones128, onesc[:, :])
            eps_sb = single.tile([128, 1], F32)
            nc.vector.memset(eps_sb, EPS)
            g0_sb = single.tile([128, 2], F32)
            nc.sync.dma_start(g0_sb, g0c[:, :])
            c0_sb = single.tile([128, 2], F32)
            nc.sync.dma_start(c0_sb, c0c[:, :])
            dtb_sb = single.tile([128, NCH], F32)
            nc.sync.dma_start(dtb_sb, dtbc[:, :])
            a_sb = single.tile([128, NCH, N], F32)
            nc.sync.dma_start(a_sb, acols[:, :, :])
            w_sb = single.tile([128, NCH], F32)
            nc.sync.dma_start(w_sb, wbc[:, :])
            b_sb = single.tile([128, NCH], F32)
            nc.sync.dma_start(b_sb, bbc[:, :])
            dp_sb = single.tile([128, NCH], F32)
            nc.sync.dma_start(dp_sb, dpc[:, :])
            dtw_sb = single.tile([128, DQ], F16)
            nc.sync.dma_start(dtw_sb, dtwT[:, :])
            id16 = single.tile([128, 128], F16)
            nc.sync.dma_start(id16, idmc[:, :])

            hcarry = hcp.tile([128, NCH, NGRP, G], F32)
            nc.vector.memset(hcarry, 0.0)

            for q in range(NQ):
                t0q = q * LQ
                # ---------------- phase A ----------------
                delta = qpool.tile([128, NCH, SEG], F16, tag="delta")
                dxn = qpool.tile([128, NCH, LQ], F16, tag="dxn")
                for c in range(NCH):
                    nc.vector.memset(delta[:, c, 0:2], 60000.0)
                for itb in range(NTB):
                    t0 = t0q + itb * TB
                    dts = slice(2 + itb * TB, 2 + (itb + 1) * TB)
                    xts = slice(itb * TB, (itb + 1) * TB)
                    ps_sx = psum.tile([128, TB], F32, tag="ps_sx")
                    ps_sxx = psum.tile([128, TB], F32, tag="ps_sxx")
                    ps_g0 = psum.tile([128, TB], F32, tag="ps_g0")
                    ps_g1 = psum.tile([128, TB], F32, tag="ps_g1")
                    own_tiles = {}
                    for ch in range(NCHALL):
                        if ch < NCH:  # own-quarter chunk (host row-permutation)
                            xc = xown.tile([128, TB], F16, tag="xown")
                            own_tiles[ch] = xc
                        else:
                            xc = xin.tile([128, TB], F16, tag="xin")
                        nc.sync.dma_start(xc, xT[ch * 128 : (ch + 1) * 128, t0 : t0 + TB])
                        x2 = wrk.tile([128, TB], F16, tag="x2", bufs=2)
                        nc.scalar.square(x2, xc)
                        st = ch == 0
                        sp = ch == NCHALL - 1
                        nc.tensor.matmul(ps_sx, ones128, xc, start=st, stop=sp)
                        nc.tensor.matmul(ps_sxx, ones128, x2, start=st, stop=sp)
                        wx = wpool.tile([128, P_PROJ], F16, tag="wx")
                        nc.sync.dma_start(wx, wxF[ch * 128 : (ch + 1) * 128, :])
                        nc.tensor.matmul(ps_g0, wx[:, 0:128], xc, start=st, stop=sp)
                        nc.tensor.matmul(ps_g1, wx[:, 128:256], xc, start=st, stop=sp)

                    # stats: mean, rstd (all partition rows carry the same value)
                    mean_b = stats.tile([128, TB], F32, tag="mean")
                    nc.scalar.mul(mean_b, ps_sx, 1.0 / D)
                    msq = stats.tile([128, TB], F32, tag="sA")
                    nc.scalar.mul(msq, ps_sxx, 1.0 / D)
                    m2 = wrk.tile([128, TB], F32, tag="wa")
                    nc.vector.tensor_mul(m2, mean_b, mean_b)
                    nc.vector.tensor_sub(msq, msq, m2)  # msq <- var
                    # rstd = exp(-0.5*ln(var+eps))
                    nc.scalar.activation(m2, msq, AF.Ln, bias=eps_sb[:, 0:1])
                    rstd_b = stats2.tile([128, TB], F32, tag="rstd")
                    nc.scalar.activation(rstd_b, m2, AF.Exp, scale=-0.5)
                    mr_b = stats2.tile([128, TB], F32, tag="mr")
                    nc.vector.tensor_mul(mr_b, mean_b, rstd_b)

                    # proj = rstd*G - (mr*g0 - c0)   (LN folded into x_proj)
                    dr_sb = wrk.tile([128, TB], F16, tag="drt", bufs=2)
                    bc_blk = wrk.tile([128, TB], F16, tag="bcb", bufs=2)
                    for ph, ps_g in enumerate([ps_g0, ps_g1]):
                        s1 = wrk.tile([128, TB], F32, tag="wb")
                        nc.vector.tensor_mul(s1, ps_g, rstd_b)
                        s2 = wrk.tile([128, TB], F32, tag="wc")
                        nc.vector.tensor_scalar(
                            s2,
                            mr_b,
                            g0_sb[:, ph : ph + 1],
                            c0_sb[:, ph : ph + 1],
                            op0=ALU.mult,
                            op1=ALU.subtract,
                        )
                        tgt = dr_sb if ph == 0 else bc_blk
                        nc.vector.tensor_sub(tgt, s1, s2)
                    nc.sync.dma_start(bcscr[:, t0 : t0 + TB], bc_blk)

                    # dt_proj + softplus -> delta; xn; dxn = delta*xn
                    for c in range(NCH):
                        ps_dt = psum.tile([128, TB], F32, tag="ps_sx")
                        nc.tensor.matmul(
                            ps_dt,
                            dtw_sb[:, c * 128 : (c + 1) * 128],
                            dr_sb,
                            start=True,
                            stop=True,
                        )
                        # softplus(z) = relu(z) + ln(1 + exp(-|z|))
                        dsl = delta[:, c, dts]
                        t_abs = wrk.tile([128, TB], F32, tag="wa")
                        nc.scalar.activation(
                            t_abs, ps_dt, AF.Abs, bias=dtb_sb[:, c : c + 1]
                        )
                        nc.scalar.activation(t_abs, t_abs, AF.Exp, scale=-1.0)
                        nc.scalar.activation(t_abs, t_abs, AF.Ln, bias=1.0)
                        t_r = wrk.tile([128, TB], F32, tag="wb")
                        nc.scalar.activation(
                            t_r, ps_dt, AF.Relu, bias=dtb_sb[:, c : c + 1]
                        )
                        nc.vector.tensor_add(dsl, t_abs, t_r)
                        xc = own_tiles[c]
                        t1 = wrk.tile([128, TB], F32, tag="wa2")
                        nc.vector.tensor_mul(t1, xc, rstd_b)
                        nc.vector.tensor_sub(t1, t1, mr_b)
                        xnc = wrk.tile([128, TB], F32, tag="wc2")
                        nc.scalar.activation(
                            xnc,
                            t1,
                            AF.Identity,
                            bias=b_sb[:, c : c + 1],
                            scale=w_sb[:, c : c + 1],
                        )
                        nc.vector.tensor_mul(dxn[:, c, xts], dsl, xnc)

                # ---------------- phase B ----------------
                # z_c starts as D_param*x; group partials of sum_n C_n*h_n
                # are accumulated into it (f16) by gpsimd.
                zs = []
                for c in range(NCH):
                    xr = cpool.tile([128, LQ], F16, tag="xr", bufs=2)
                    nc.scalar.dma_start(
                        xr, xT[c * 128 : (c + 1) * 128, t0q : t0q + LQ]
                    )
                    zc = zpool.tile([128, LQ], F16, tag=f"z{c}")
                    nc.vector.tensor_scalar_mul(zc, xr, dp_sb[:, c : c + 1])
                    zs.append(zc)
                for grp in range(NGRP):
                    n0 = grp * G
                    bB = bcBp.tile([128, G, LQ], F16, tag="bB")
                    cB = bcCp.tile([128, G, LQ], F16, tag="cB")
                    for g in range(G):
                        nc.sync.dma_start(
                            bB[:, g, :],
                            bcscr[n0 + g : n0 + g + 1, t0q : t0q + LQ].to_broadcast(
                                [128, LQ]
                            ),
                        )
                        nc.sync.dma_start(
                            cB[:, g, :],
                            bcscr[
                                64 + n0 + g : 65 + n0 + g, t0q : t0q + LQ
                            ].to_broadcast([128, LQ]),
                        )
                    for c in range(NCH):
                        dA = bigA.tile([128, G, SEG], F16, tag="dA")
                        uB = bigU.tile([128, G, SEG], F16, tag="u")
                        hB = bigH.tile([128, G, SEG], F16, tag="h")
                        for g in range(G):
                            nc.scalar.activation(
                                dA[:, g, :],
                                delta[:, c, :],
                                AF.Exp,
                                scale=a_sb[:, c, n0 + g : n0 + g + 1],
                            )
                            nc.vector.tensor_mul(
                                uB[:, g, 2:SEG], dxn[:, c, :], bB[:, g, :]
                            )
                        hcs = hcarry[:, c, grp, :]
                        nc.vector.tensor_copy(uB[:, :, 0], hcs)
                        nc.vector.tensor_copy(uB[:, :, 1], hcs)
                        nc.vector.tensor_tensor_scan(
                            hB[:, :, :].rearrange("p a b -> p (a b)"),
                            dA[:, :, :].rearrange("p a b -> p (a b)"),
                            uB[:, :, :].rearrange("p a b -> p (a b)"),
                            0.0,
                            op0=ALU.mult,
                            op1=ALU.add,
                        )
                        if q < NQ - 1:
                            nc.gpsimd.tensor_copy(hcs, hB[:, :, SEG - 1])
                        ps_y0 = psum.tile([128, TB], F32, tag="ps_y0")
                        ps_y1 = psum.tile([128, TB], F32, tag="ps_y1")
                        for g in range(G):
                            nc.vector.tensor_mul(
                                hB[:, g, 2:SEG], hB[:, g, 2:SEG], cB[:, g, :]
                            )
                            st = g == 0
                            sp = g == G - 1
                            nc.tensor.matmul(
                                ps_y0, id16, hB[:, g, 2 : 2 + TB],
                                start=st, stop=sp,
                            )
                            nc.tensor.matmul(
                                ps_y1, id16, hB[:, g, 2 + TB : SEG],
                                start=st, stop=sp,
                            )
                        gp = gpp.tile([128, LQ], F16, tag="gp")
                        nc.scalar.copy(gp[:, 0:TB], ps_y0)
                        nc.scalar.copy(gp[:, TB:LQ], ps_y1)
                        nc.vector.tensor_add(zs[c], zs[c], gp)

                # ---------------- phase C: out_proj partial ----------------
                for itb in range(NTB):
                    t0 = t0q + itb * TB
                    zbs = [zs[c][:, itb * TB : (itb + 1) * TB] for c in range(NCH)]
                    for o in range(NCHALL):
                        ps_o = psum.tile(
                            [128, TB], F32, tag="ps_o0" if o % 2 == 0 else "ps_o1"
                        )
                        for c in range(NCH):
                            wo = wopool.tile([128, 128], F16, tag="wo")
                            nc.gpsimd.dma_start(
                                wo,
                                woT[c * 128 : (c + 1) * 128, o * 128 : (o + 1) * 128],
                            )
                            nc.tensor.matmul(
                                ps_o, wo, zbs[c], start=(c == 0), stop=(c == NCH - 1)
                            )
                        ostg = cpool.tile([128, TB], F32, tag="ostg", bufs=1)
                        nc.scalar.copy(ostg, ps_o)
                        nc.sync.dma_start(
                            out_part[o * 128 : (o + 1) * 128, t0 : t0 + TB], ostg
                        )
    return nc


def _get_program(L_):
    if L_ not in _PROGRAM_CACHE:
        import concourse.bacc as bacc

        nc = bacc.Bacc(None, target_bir_lowering=False)
        _build(nc, L_)
        nc.compile()
        _PROGRAM_CACHE[L_] = nc
    return _PROGRAM_CACHE[L_]


def _cols(v):
    """[DQ] -> [128, NCH] per-partition column layout (chunk-major)."""
    return np.ascontiguousarray(v.reshape(NCH, 128).T).astype(np.float32)


HW_EXEC_NS = None
_NEFF_CACHE = {}


def _profiled_run(nc, in_maps):
    """Run via PJRT with the terminal-side NRT profiler capturing NTFFs,
    then extract device exec time with neuron-profile. Falls back to an
    unprofiled run on any failure."""
    global HW_EXEC_NS
    import glob as globmod
    import json
    import subprocess
    import tempfile
    from dataclasses import dataclass

    from concourse import bass2jax

    try:
        sys.path.insert(0, "/root/.axon_site")
        from trn_agent_boot.trn_boot import _ntff_profile_via_ctypes

        hook = _ntff_profile_via_ctypes("/opt/axon/libaxon_pjrt.so")
        assert hook is not None
        neff_dir = tempfile.mkdtemp(prefix="ssmprof_")
        with hook(neff_dir, [0]):
            results = bass2jax.run_bass_via_pjrt(nc, in_maps, n_cores=8)
        ntffs = sorted(globmod.glob(os.path.join(neff_dir, "*.ntff")))
        if not ntffs:
            print("profiling: no NTFF captured")
        else:
            neffs = sorted(globmod.glob(os.path.join(neff_dir, "*.neff")))
            neff = neffs[0]
            out_json = os.path.join(neff_dir, "prof.json")
            subprocess.run(
                ["neuron-profile", "view", "-n", neff, "-s", ntffs[0],
                 "--output-format=json", "--output-file", out_json,
                 "--ignore-nc-buf-usage"],
                check=True, env=dict(os.environ, NEURON_PROFILE_DBG_OUTPUT="2"),
                capture_output=True, text=True,
            )
            with open(out_json) as f:
                prof = json.load(f)
            insts = prof.get("instruction", [])
            if insts:
                t0 = min(i["timestamp"] for i in insts)
                t1 = max(i["timestamp"] + i.get("duration", 0) for i in insts)
                HW_EXEC_NS = int(t1 - t0)
            else:
                summ = prof.get("summary", {})
                HW_EXEC_NS = summ.get("total_time_ns")
            print(f"profiled exec: {HW_EXEC_NS} ns; json: {out_json}")

        @dataclass
        class _R:
            results: list
            exec_time_ns: object
            instructions_and_trace: object = None

        return _R(results=results, exec_time_ns=HW_EXEC_NS)
    except Exception as e:
        print(f"profiling failed ({type(e).__name__}: {e}); plain run")
        from concourse.bass_utils import run_bass_kernel_spmd as _run

        return _run(nc, in_maps, core_ids=list(range(8)), trace=False)


def kernel(
    x, norm_w, norm_b, x_proj_w, dt_proj_w, dt_proj_b, A_log, D_param, out_proj_w
):
    global LAST_RESULTS
    L_ = x.shape[1]
    nc = _get_program(L_)

    # host-side weight prep (small tensors only)
    wxF = (norm_w[:, None] * x_proj_w.T).astype(np.float32)  # [D, 256]
    g0 = (norm_w @ x_proj_w.T).astype(np.float32)  # [256]
    c0 = (norm_b @ x_proj_w.T).astype(np.float32)
    g0c = np.ascontiguousarray(g0.reshape(2, 128).T).astype(np.float32)
    c0c = np.ascontiguousarray(c0.reshape(2, 128).T).astype(np.float32)
    A = (-np.exp(A_log.astype(np.float64))).astype(np.float32)  # [D, N]
    dtwT_full = np.ascontiguousarray(dt_proj_w.T).astype(np.float32)  # [R, D]
    woT_full = np.ascontiguousarray(out_proj_w.T).astype(np.float32)  # [D, D]

    in_maps = []
    for core in range(8):
        b, qq = core // 4, core % 4
        sl = slice(DQ * qq, DQ * (qq + 1))
        own = np.arange(DQ * qq, DQ * (qq + 1))
        perm = np.concatenate([own, np.delete(np.arange(D), own)])
        acols = np.ascontiguousarray(
            A[sl].reshape(NCH, 128, N).transpose(1, 0, 2)
        ).astype(np.float32)
        in_maps.append(
            {
                "xT": np.ascontiguousarray(x[b].T[perm]).astype(np.float16),
                "wxF": np.ascontiguousarray(wxF[perm]).astype(np.float16),
                "g0c": g0c,
                "c0c": c0c,
                "dtwT": np.ascontiguousarray(dtwT_full[:, sl]).astype(np.float16),
                "dtbc": _cols(dt_proj_b[sl]),
                "acols": acols,
                "wbc": _cols(norm_w[sl]),
                "bbc": _cols(norm_b[sl]),
                "dpc": _cols(D_param[sl]),
                "woT": np.ascontiguousarray(woT_full[sl]).astype(np.float16),
                "onesc": np.ones((128, 128), np.float16),
                "idmc": np.eye(128, dtype=np.float16),
            }
        )

    trace = bool(int(os.environ.get("SSM_TRACE", "0")))
    if trace:
        results = _profiled_run(nc, in_maps)
        LAST_RESULTS = results
    else:
        LAST_RESULTS = run_bass_kernel_spmd(
            nc, in_maps, core_ids=list(range(8)), trace=False
        )
    parts = [r["out_part"] for r in LAST_RESULTS.results]
    out = np.stack(
        [
            (parts[0] + parts[1] + parts[2] + parts[3]).T,
            (parts[4] + parts[5] + parts[6] + parts[7]).T,
        ]
    ).astype(np.float32)
    return out
